# revision 14
# baseline (speedup 1.0000x reference)
"""Trainium2 Bass kernel for ALayer: out = x * box3x3(sigmoid(conv3x3(relu(conv3x3(x,w1)),w2))).

Sharding: pure data parallel over batch (32 images -> 4 per core x 8 cores).

v2 redesign vs v1 baseline (247us):
  - edge columns handled by batched 32-row edge matmuls (3 MMs/block) instead
    of ~384 tiny per-16-row matmuls (each tiny MM costs a flat ~227ns on PE)
  - startup memsets reduced to guard-only (v1 spent ~30us memsetting A2 while
    every other engine idled)
  - box+broadcast matmul switched from DoubleRow (streams 2N cols) to plain
    fp8 K=36 (streams N cols) with single-segment A2 replica layout
  - eviction chains spread S(ACT copy, psum) -> V(add, psum) -> G(relu, sbuf)
  - final multiply writes a separate bf16 out buffer (x_sb stays pristine;
    32-row SWDGE cast stores, 8 total), mul work split V / S+G
  - loads in 16 one-MB chunks, repl DMAs at 32-row granularity
"""

import numpy as np
import ml_dtypes

import concourse.bacc as bacc
import concourse.mybir as mybir
from concourse.tile import TileContext
from concourse.bass_utils import run_bass_kernel_spmd
from concourse.ap import AP

BF16 = mybir.dt.bfloat16
F32 = mybir.dt.float32
FP8 = mybir.dt.float8e4
AF = mybir.ActivationFunctionType

# Problem constants (hardcoded; kernel.py must be self-contained)
B, C, H, W = 32, 64, 128, 128
N_CORES = 8
B_LOC = B // N_CORES            # 4 images per core
PACKS = 2
S = H * W                       # 16384
FG = 160                        # front zero-guard
BG = 288
BUF_W = FG + S + BG             # 16832
TAPS = (4, 0, 1, 2, 3, 5, 6, 7, 8)   # A2 slot -> tap index (center first)
KXM = (1, 2, 0)                 # edge psum block -> kx


def _pos(y, x):
    return FG + y * W + x


def _host_weights(w1, w2):
    w1 = np.asarray(w1, np.float32)     # [16, 64, 3, 3]
    w2 = np.asarray(w2, np.float32)     # [1, 16, 3, 3]
    bf = ml_dtypes.bfloat16
    f8 = mybir.dt.np(FP8)

    # conv1 DoubleRow M=128: psum m = dx*64 + pk*32 + il*16 + co; dx in {0,1}
    # (w1A) and dx=2 (w1B, M=64) overlaid into the dx=0 block reading rhs +2:
    # h[p] = relu(psum0[p-1] + psum1[p]).
    w1A = np.zeros((128, 2, 3, 128), np.float32)
    w1B = np.zeros((128, 2, 3, 64), np.float32)
    for il in range(2):
        for pk in range(2):
            for ky in range(3):
                for dx in range(2):
                    m0 = dx * 64 + pk * 32 + il * 16
                    w1A[il * 64:(il + 1) * 64, pk, ky, m0:m0 + 16] = \
                        w1[:, :, ky, dx].T
                m0 = pk * 32 + il * 16
                w1B[il * 64:(il + 1) * 64, pk, ky, m0:m0 + 16] = \
                    w1[:, :, ky, 2].T
    # conv1 edges (bf16, per pack, 3 kx-blocks):
    # m = blk*32 + il*16 + co with blk kx map KXM
    w1E = np.zeros((128, 3, 96), np.float32)
    for il in range(2):
        for ky in range(3):
            for blk in range(3):
                w1E[il * 64:(il + 1) * 64, ky,
                    blk * 32 + il * 16:blk * 32 + il * 16 + 16] = \
                    w1[:, :, ky, KXM[blk]].T
    # conv2 DoubleRow M=32 (pad; seg stride %16==0): m = 2*pk + il
    w2D = np.zeros((96, 2, 3, 32), np.float32)
    for g in range(3):
        for il in range(2):
            for pk in range(2):
                for kx in range(3):
                    w2D[g * 32 + il * 16:g * 32 + il * 16 + 16, pk, kx,
                        2 * pk + il] = w2[0, :, g, kx]
    # conv2 edges DoubleRow M=68: m = blk*32 + pk*2 + il (32-aligned blocks
    # so the eviction's psum reads start at partitions 0/32/64)
    w2E = np.zeros((96, 2, 80), np.float32)
    for g in range(3):
        for il in range(2):
            for pk in range(2):
                for blk in range(3):
                    w2E[g * 32 + il * 16:g * 32 + il * 16 + 16, pk,
                        blk * 32 + pk * 2 + il] = w2[0, :, g, KXM[blk]]
    # box weights, plain fp8 K=36: A2 row 4*s + img, m = (il, c) bcast
    onesB = np.zeros((36, 2, 128), np.float32)
    onesBE = np.zeros((36, 2, 2, 128), np.float32)
    for s, t in enumerate(TAPS):
        kx = t % 3
        for pk in range(2):
            for il in range(2):
                r = 4 * s + 2 * pk + il
                onesB[r, pk, il * 64:(il + 1) * 64] = 1.0
                if kx >= 1:
                    onesBE[r, pk, 0, il * 64:(il + 1) * 64] = 1.0
                if kx <= 1:
                    onesBE[r, pk, 1, il * 64:(il + 1) * 64] = 1.0
    return (w1A.astype(f8), w1B.astype(f8), w1E.astype(bf), w2D.astype(f8),
            w2E.astype(f8), onesB.astype(f8), onesBE.astype(f8))


def _build_nc():
    nc = bacc.Bacc(None, target_bir_lowering=False, debug=False)

    x_ext = nc.declare_dram_parameter("x", [B_LOC, C, H, W], F32, isOutput=False)
    out_ext = nc.declare_dram_parameter("out", [B_LOC, C, H, W], F32, isOutput=True)
    w1A_ext = nc.declare_dram_parameter("w1A", [128, 2, 3, 128], FP8, isOutput=False)
    w1B_ext = nc.declare_dram_parameter("w1B", [128, 2, 3, 64], FP8, isOutput=False)
    w1E_ext = nc.declare_dram_parameter("w1E", [128, 3, 96], BF16, isOutput=False)
    w2D_ext = nc.declare_dram_parameter("w2D", [96, 2, 3, 32], FP8, isOutput=False)
    w2E_ext = nc.declare_dram_parameter("w2E", [96, 2, 80], FP8, isOutput=False)
    oB_ext = nc.declare_dram_parameter("onesB", [36, 2, 128], FP8, isOutput=False)
    oBE_ext = nc.declare_dram_parameter("onesBE", [36, 2, 2, 128], FP8, isOutput=False)

    with TileContext(nc) as tc:
        with (
            tc.tile_pool(name="wpool", bufs=1) as wpool,
            tc.tile_pool(name="xpool", bufs=1) as xpool,
            tc.tile_pool(name="hpool", bufs=1) as hpool,
            tc.tile_pool(name="opool", bufs=2) as opool,
            tc.tile_pool(name="scpool", bufs=4) as scpool,
            tc.tile_pool(name="smpool", bufs=4) as smpool,
            tc.tile_pool(name="tmpool", bufs=2) as tmpool,
            tc.tile_pool(name="psA", bufs=3, space="PSUM") as psA,
            tc.tile_pool(name="psB", bufs=2, space="PSUM") as psB,
            tc.tile_pool(name="psC", bufs=2, space="PSUM") as psC,
            tc.tile_pool(name="psE", bufs=1, space="PSUM") as psE,
        ):
            w1A = wpool.tile([128, 2, 3, 128], FP8)
            w1B = wpool.tile([128, 2, 3, 64], FP8)
            w1E = wpool.tile([128, 3, 96], BF16)
            w2D = wpool.tile([96, 2, 3, 32], FP8)
            w2E = wpool.tile([96, 2, 80], FP8)
            onesB = wpool.tile([36, 2, 128], FP8)
            onesBE = wpool.tile([36, 2, 2, 128], FP8)
            for dst, src in ((w1A, w1A_ext), (w1B, w1B_ext), (w1E, w1E_ext),
                             (w2D, w2D_ext), (w2E, w2E_ext), (onesB, oB_ext),
                             (onesBE, oBE_ext)):
                nc.sync.dma_start(out=dst[:], in_=src[:])

            x_sb = [xpool.tile([128, BUF_W], BF16, name=f"xsb{p}")
                    for p in range(PACKS)]
            x8 = xpool.tile([128, 2, BUF_W], FP8, name="x8")
            h64 = hpool.tile([64, BUF_W], FP8, name="h64")
            hrep4 = hpool.tile([96, 2, BUF_W], FP8, name="hrep4")
            A2 = hpool.tile([36, BUF_W], FP8, name="A2")

            def _yx(t, off, rows, cols=2, colstep=127):
                """[P, rows@W, cols@colstep] view at flat offset `off`."""
                base = t[:, off:off + 1]
                return AP(base.tensor, base.offset,
                          [list(base.ap[0]), [W, rows], [colstep, cols]])

            def guard_memsets():
                for p in range(PACKS):
                    nc.vector.memset(x_sb[p][:, 0:FG], 0.0)
                    nc.vector.memset(x_sb[p][:, FG + S:BUF_W], 0.0)
                nc.vector.memset(x8[:, :, 0:FG], 0.0)
                nc.vector.memset(x8[:, :, FG + S:BUF_W], 0.0)
                nc.gpsimd.memset(h64[:, 0:FG], 0.0)
                nc.gpsimd.memset(h64[:, FG + S:BUF_W], 0.0)
                nc.gpsimd.memset(hrep4[:, :, 0:FG], 0.0)
                nc.gpsimd.memset(hrep4[:, :, FG + S:BUF_W], 0.0)
                nc.gpsimd.memset(A2[0:4, 0:FG], 0.0)
                nc.gpsimd.memset(A2[0:4, FG + S:BUF_W], 0.0)

            def load(p, c):
                r1 = min(16 * c + 17, H)
                nc.gpsimd.dma_start(
                    out=x_sb[p][:, _pos(16 * c, 0):_pos(r1, 0)],
                    in_=x_ext[2 * p:2 * p + 2, :, 16 * c:r1]
                    .rearrange("b c h w -> (b c) (h w)"),
                )

            def cast8(p, c):
                r1 = min(16 * c + 17, H)
                a, b = _pos(16 * c, 0), _pos(r1, 0)
                k = (2 * c + p) % 3
                if k == 0:
                    nc.scalar.activation(x8[:, p, a:b], x_sb[p][:, a:b], AF.Copy)
                elif k == 1:
                    nc.vector.tensor_copy(x8[:, p, a:b], x_sb[p][:, a:b])
                else:
                    nc.gpsimd.tensor_copy(x8[:, p, a:b], x_sb[p][:, a:b])

            def conv1(v):
                # 16 rows = 4 psum tiles; weight reuse across pairs of tiles
                for g in range(2):
                    rts = (4 * v + 2 * g, 4 * v + 2 * g + 1)
                    pas = {rt: psA.tile([128, 512], F32, tag="psa", name="pa")
                           for rt in rts}
                    # w1A ky0 first (start clears the bank), w1B accumulates
                    # into [0:64], w1A ky1/ky2 last so stop closes the group
                    for rt in rts:
                        q = _pos(rt * 4 - 1, 0)
                        nc.tensor.matmul(
                            pas[rt][:, :], w1A[:, :, 0, :],
                            x8[:, :, q:q + 512],
                            perf_mode=mybir.MatmulPerfMode.DoubleRow,
                            start=True, stop=False)
                    for ky in range(3):
                        for rt in rts:
                            q = _pos(rt * 4 + ky - 1, 2)
                            nc.tensor.matmul(
                                pas[rt][0:64, 0:510], w1B[:, :, ky, :],
                                x8[:, :, q:q + 510],
                                perf_mode=mybir.MatmulPerfMode.DoubleRow,
                                start=False, stop=False)
                    for ky in (1, 2):
                        for rt in rts:
                            q = _pos(rt * 4 + ky - 1, 0)
                            nc.tensor.matmul(
                                pas[rt][:, :], w1A[:, :, ky, :],
                                x8[:, :, q:q + 512],
                                perf_mode=mybir.MatmulPerfMode.DoubleRow,
                                start=False, stop=(ky == 2))
                    for rt in rts:
                        q0 = _pos(rt * 4, 0)
                        pa = pas[rt]
                        t1 = scpool.tile([64, 512], BF16, tag="sc", name="t1")
                        nc.scalar.activation(t1[:, 0:510], pa[64:128, 1:511],
                                             AF.Copy)
                        nc.vector.tensor_add(t1[:, 0:510], pa[0:64, 0:510],
                                             t1[:, 0:510])
                        # interior columns only: edge cols are owned by
                        # conv1_edges (which may have run already)
                        hdst = AP(h64.tensor, q0 + 1,
                                  [list(h64[:, 0:1].ap[0]), [W, 4], [1, 126]])
                        tsrc = AP(t1.tensor, t1[:, 0:1].offset,
                                  [list(t1[:, 0:1].ap[0]), [W, 4], [1, 126]])
                        nc.gpsimd.tensor_scalar_max(hdst, tsrc, 0.0)

            def conv1_edges(bb, p):
                # 32 rows, both sides: 3 bf16 MMs, K=128, M=96, N=128
                # rhs cols per row: {0, 1, 126, 127}; psum col j = 4y+u*2+v
                pe1 = psE.tile([96, 128], F32, tag="pse", name="pe1")
                for ky in range(3):
                    base = x_sb[p][:, _pos(32 * bb + ky - 1, 0):
                                   _pos(32 * bb + ky - 1, 0) + 1]
                    rhs = AP(base.tensor, base.offset,
                             [list(base.ap[0]), [W, 32], [126, 2], [1, 2]])
                    nc.tensor.matmul(pe1[:, :], w1E[:, ky, :], rhs,
                                     start=(ky == 0), stop=(ky == 2))
                # left[y] = blk0[4y] + blk1[4y+1]; right[y] = blk2[4y+2] + blk0[4y+3]
                te1 = smpool.tile([32, 128], BF16, tag="sm", name="te1")
                ts1 = smpool.tile([32, 32, 2], BF16, tag="sm", name="ts1")
                nc.scalar.activation(te1[:, :], pe1[0:32, :], AF.Copy)
                te1v = te1.rearrange("p (y r) -> p y r", r=4)
                pe1v1 = pe1[32:64, :].rearrange("p (y r) -> p y r", r=4)
                pe1v2 = pe1[64:96, :].rearrange("p (y r) -> p y r", r=4)
                nc.vector.tensor_add(ts1[:, :, 0], te1v[:, :, 0], pe1v1[:, :, 1])
                nc.vector.tensor_add(ts1[:, :, 1], te1v[:, :, 3], pe1v2[:, :, 2])
                nc.gpsimd.tensor_scalar_max(
                    _yx(h64[32 * p:32 * p + 32], _pos(32 * bb, 0), 32),
                    ts1[:, :, :], 0.0)

            def repl_h(vv):
                a = max(0, _pos(32 * vv, 0) - W - 4)
                b = min(BUF_W, _pos(32 * vv + 32, 0) + W + 4)
                for s in range(2):
                    src = h64[32 * s:32 * s + 32]
                    a0 = max(a, W)
                    nc.sync.dma_start(out=hrep4[0:32, s, a0:b],
                                      in_=src[:, a0 - W:b - W])
                    nc.sync.dma_start(out=hrep4[32:64, s, a:b],
                                      in_=src[:, a:b])
                    b2 = min(b, BUF_W - W)
                    nc.sync.dma_start(out=hrep4[64:96, s, a:b2],
                                      in_=src[:, a + W:b2 + W])

            def conv2(v):
                for rt in range(4 * v, 4 * v + 4):
                    pz = psC.tile([32, 512], F32, tag="psc", name="pz")
                    for kx in range(3):
                        q = _pos(rt * 4, kx - 1)
                        nc.tensor.matmul(
                            pz[:, :], w2D[:, :, kx, :],
                            hrep4[:, :, q:q + 512],
                            perf_mode=mybir.MatmulPerfMode.DoubleRow,
                            start=(kx == 0), stop=(kx == 2))
                    q0 = _pos(rt * 4, 0)
                    nc.scalar.activation(A2[0:4, q0:q0 + 512], pz[0:4, :],
                                         AF.Sigmoid)

            def conv2_edges(bb):
                # one DR MM: K=192, M=12, rhs cols {0,1,126,127} x 32 rows
                pe2 = psE.tile([80, 4, 32], F32, tag="pse", name="pe2")
                for j, col in enumerate((0, 1, 126, 127)):
                    base = hrep4[:, 0, _pos(32 * bb, col):
                                 _pos(32 * bb, col) + 1]
                    rhs = AP(base.tensor, base.offset,
                             [list(base.ap[0]), [BUF_W, 2], [W, 32]])
                    nc.tensor.matmul(pe2[:, j, :], w2E[:, :, :], rhs,
                                     perf_mode=mybir.MatmulPerfMode.DoubleRow,
                                     start=(j == 0), stop=(j == 3))
                te2 = smpool.tile([4, 4, 32], BF16, tag="sm", name="te2")
                ts2 = smpool.tile([4, 32, 2], BF16, tag="sm", name="ts2")
                nc.scalar.activation(te2[:, :, :], pe2[0:4, :, :], AF.Copy)
                nc.vector.tensor_add(ts2[:, :, 0], te2[:, 0, :], pe2[32:36, 1, :])
                nc.vector.tensor_add(ts2[:, :, 1], te2[:, 3, :], pe2[64:68, 2, :])
                nc.scalar.activation(_yx(A2[0:4], _pos(32 * bb, 0), 32),
                                     ts2[:, :, :], AF.Sigmoid)

            def repl_a(bb):
                lo = max(0, _pos(32 * bb, 0) - 132)
                hi = _pos(32 * bb + 32, 0) if bb < 3 else FG + S
                for s in range(1, 9):
                    t = TAPS[s]
                    o = (t // 3 - 1) * W + (t % 3 - 1)
                    a = max(lo, -o)
                    nc.sync.dma_start(out=A2[4 * s:4 * s + 4, a:hi],
                                      in_=A2[0:4, a + o:hi + o])

            outs = {}

            def box(v):
                vv = v // 2
                if (vv, 0) not in outs:
                    for p in range(PACKS):
                        outs[(vv, p)] = opool.tile([128, 32 * W], BF16,
                                                   tag="ob", name=f"ob{p}")
                for p in range(PACKS):
                    o_sb = outs[(vv, p)]
                    for rt in range(4 * v, 4 * v + 4):
                        q0 = _pos(rt * 4, 0)
                        oo = (rt - 8 * vv) * 512
                        pb = psB.tile([128, 512], F32, tag="psb", name="pb")
                        nc.tensor.matmul(pb[:, :], onesB[:, p, :],
                                         A2[0:36, q0:q0 + 512],
                                         start=True, stop=True)
                        if rt % 2 == 0:
                            nc.vector.tensor_mul(
                                o_sb[:, oo:oo + 512],
                                x_sb[p][:, q0:q0 + 512], pb[:, :])
                        else:
                            tm = tmpool.tile([128, 512], BF16, tag="tm",
                                             name="tm")
                            nc.scalar.activation(tm[:, :], pb[:, :], AF.Copy)
                            nc.gpsimd.tensor_mul(
                                o_sb[:, oo:oo + 512],
                                x_sb[p][:, q0:q0 + 512], tm[:, :])

            def box_edges(vv):
                for p in range(PACKS):
                    pbe = psE.tile([128, 2, 32], F32, tag="pse", name="pbe")
                    for e, col in enumerate((0, W - 1)):
                        base = A2[0:36, _pos(32 * vv, col):
                                  _pos(32 * vv, col) + 1]
                        rhs = AP(base.tensor, base.offset,
                                 [list(base.ap[0]), [W, 32]])
                        nc.tensor.matmul(pbe[:, e, :], onesBE[:, p, e, :],
                                         rhs, start=(e == 0), stop=(e == 1))
                    o_sb = outs[(vv, p)]
                    dst = o_sb[:, 0:32 * W].rearrange(
                        "p (y x) -> p y x", x=W)[:, :, 0:W:W - 1]
                    nc.vector.tensor_mul(
                        dst, _yx(x_sb[p], _pos(32 * vv, 0), 32),
                        pbe[:, :, :].rearrange("p e y -> p y e"))

            def store(vv, p):
                nc.gpsimd.dma_start(
                    out=out_ext[2 * p:2 * p + 2, :, 32 * vv:32 * vv + 32]
                    .rearrange("b c h w -> (b c) (h w)"),
                    in_=outs[(vv, p)][:, :],
                )

            # ---- emission: 16-row ticks ----
            guard_memsets()
            for c in (0, 1, 2):
                for p in range(PACKS):
                    load(p, c)
                    cast8(p, c)
            for u in range(1, 14):
                c = u + 2
                if c < 8:
                    for p in range(PACKS):
                        load(p, c)
                        cast8(p, c)
                if u % 2 == 1 and u <= 7:
                    bb = (u - 1) // 2
                    conv1_edges(bb, 0)
                    conv1_edges(bb, 1)
                v = u - 1
                if 0 <= v < 8:
                    conv1(v)
                if u % 2 == 1 and 3 <= u <= 9:
                    repl_h((u - 3) // 2)
                v = u - 3
                if 0 <= v < 8:
                    conv2(v)
                if u % 2 == 0 and 4 <= u <= 10:
                    conv2_edges((u - 4) // 2)
                if u % 2 == 0 and 6 <= u <= 12:
                    bb = (u - 6) // 2
                    repl_a(bb)
                    box(2 * bb)
                    box(2 * bb + 1)
                    box_edges(bb)
                if u % 2 == 1 and 7 <= u <= 13:
                    vv = (u - 7) // 2
                    store(vv, 0)
                    store(vv, 1)

    nc.compile()
    return nc


_CACHE = {}


def _get_nc():
    if "nc" not in _CACHE:
        _CACHE["nc"] = _build_nc()
    return _CACHE["nc"]


def _reset_device():
    try:
        import ctypes

        lib = ctypes.CDLL("/opt/axon/libaxon_pjrt.so")
        lib.axon_reset.restype = ctypes.c_int64
        lib.axon_reset()
    except Exception:
        pass


def _run(x, w1, w2, trace=False):
    x = np.ascontiguousarray(np.asarray(x, np.float32))
    w1A, w1B, w1E, w2D, w2E, onesB, onesBE = _host_weights(w1, w2)
    nc = _get_nc()
    in_maps = []
    for k in range(N_CORES):
        in_maps.append({
            "x": x[k * B_LOC:(k + 1) * B_LOC],
            "w1A": w1A, "w1B": w1B, "w1E": w1E, "w2D": w2D, "w2E": w2E,
            "onesB": onesB, "onesBE": onesBE,
        })
    try:
        res = run_bass_kernel_spmd(nc, in_maps, core_ids=list(range(N_CORES)),
                                   trace=trace)
    except Exception as e:
        if "unrecoverable" not in str(e).lower():
            raise
        _reset_device()
        res = run_bass_kernel_spmd(nc, in_maps, core_ids=list(range(N_CORES)),
                                   trace=trace)
    out = np.concatenate([r["out"] for r in res.results], axis=0)
    return out.astype(np.float32), res


def kernel(x, weights, w1, w2):
    out, _ = _run(x, w1, w2, trace=False)
    return out


def kernel_timed(x, weights, w1, w2):
    out, res = _run(x, w1, w2, trace=True)
    return out, res.exec_time_ns


# revision 15
# speedup vs baseline: 1.7877x; 1.7877x over previous
"""Trainium2 Bass kernel for ALayer: out = x * box3x3(sigmoid(conv3x3(relu(conv3x3(x,w1)),w2))).

Sharding: pure data parallel over batch (32 images -> 4 per core x 8 cores).

v2 redesign vs v1 baseline (247us):
  - edge columns handled by batched 32-row edge matmuls (3 MMs/block) instead
    of ~384 tiny per-16-row matmuls (each tiny MM costs a flat ~227ns on PE)
  - startup memsets reduced to guard-only (v1 spent ~30us memsetting A2 while
    every other engine idled)
  - box+broadcast matmul switched from DoubleRow (streams 2N cols) to plain
    fp8 K=36 (streams N cols) with single-segment A2 replica layout
  - eviction chains spread S(ACT copy, psum) -> V(add, psum) -> G(relu, sbuf)
  - final multiply writes a separate bf16 out buffer (x_sb stays pristine;
    32-row SWDGE cast stores, 8 total), mul work split V / S+G
  - loads in 16 one-MB chunks, repl DMAs at 32-row granularity
"""

import numpy as np
import ml_dtypes

import concourse.bacc as bacc
import concourse.mybir as mybir
from concourse.tile import TileContext
from concourse.bass_utils import run_bass_kernel_spmd
from concourse.ap import AP

BF16 = mybir.dt.bfloat16
F32 = mybir.dt.float32
FP8 = mybir.dt.float8e4
AF = mybir.ActivationFunctionType

# Problem constants (hardcoded; kernel.py must be self-contained)
B, C, H, W = 32, 64, 128, 128
N_CORES = 8
B_LOC = B // N_CORES            # 4 images per core
PACKS = 2
S = H * W                       # 16384
FG = 160                        # front zero-guard
BG = 288
BUF_W = FG + S + BG             # 16832
TAPS = (4, 0, 1, 2, 3, 5, 6, 7, 8)   # A2 slot -> tap index (center first)
KXM = (1, 2, 0)                 # edge psum block -> kx


def _pos(y, x):
    return FG + y * W + x


def _host_weights(w1, w2):
    w1 = np.asarray(w1, np.float32)     # [16, 64, 3, 3]
    w2 = np.asarray(w2, np.float32)     # [1, 16, 3, 3]
    bf = ml_dtypes.bfloat16
    f8 = mybir.dt.np(FP8)

    # conv1 DoubleRow M=128: psum m = dx*64 + pk*32 + il*16 + co; dx in {0,1}
    # (w1A) and dx=2 (w1B, M=64) overlaid into the dx=0 block reading rhs +2:
    # h[p] = relu(psum0[p-1] + psum1[p]).
    w1A = np.zeros((128, 2, 3, 128), np.float32)
    w1B = np.zeros((128, 2, 3, 64), np.float32)
    for il in range(2):
        for pk in range(2):
            for ky in range(3):
                for dx in range(2):
                    m0 = dx * 64 + pk * 32 + il * 16
                    w1A[il * 64:(il + 1) * 64, pk, ky, m0:m0 + 16] = \
                        w1[:, :, ky, dx].T
                m0 = pk * 32 + il * 16
                w1B[il * 64:(il + 1) * 64, pk, ky, m0:m0 + 16] = \
                    w1[:, :, ky, 2].T
    # conv1 edges (bf16, per pack, 3 kx-blocks):
    # m = blk*32 + il*16 + co with blk kx map KXM
    w1E = np.zeros((128, 3, 96), np.float32)
    for il in range(2):
        for ky in range(3):
            for blk in range(3):
                w1E[il * 64:(il + 1) * 64, ky,
                    blk * 32 + il * 16:blk * 32 + il * 16 + 16] = \
                    w1[:, :, ky, KXM[blk]].T
    # conv2 DoubleRow M=32 (pad; seg stride %16==0): m = 2*pk + il
    w2D = np.zeros((96, 2, 3, 32), np.float32)
    for g in range(3):
        for il in range(2):
            for pk in range(2):
                for kx in range(3):
                    w2D[g * 32 + il * 16:g * 32 + il * 16 + 16, pk, kx,
                        2 * pk + il] = w2[0, :, g, kx]
    # conv2 edges DoubleRow M=68: m = blk*32 + pk*2 + il (32-aligned blocks
    # so the eviction's psum reads start at partitions 0/32/64)
    w2E = np.zeros((96, 2, 80), np.float32)
    for g in range(3):
        for il in range(2):
            for pk in range(2):
                for blk in range(3):
                    w2E[g * 32 + il * 16:g * 32 + il * 16 + 16, pk,
                        blk * 32 + pk * 2 + il] = w2[0, :, g, KXM[blk]]
    # box weights, plain fp8 K=36: A2 row 4*s + img, m = (il, c) bcast
    onesB = np.zeros((36, 2, 128), np.float32)
    onesBE = np.zeros((36, 2, 2, 128), np.float32)
    for s, t in enumerate(TAPS):
        kx = t % 3
        for pk in range(2):
            for il in range(2):
                r = 4 * s + 2 * pk + il
                onesB[r, pk, il * 64:(il + 1) * 64] = 1.0
                if kx >= 1:
                    onesBE[r, pk, 0, il * 64:(il + 1) * 64] = 1.0
                if kx <= 1:
                    onesBE[r, pk, 1, il * 64:(il + 1) * 64] = 1.0
    return (w1A.astype(f8), w1B.astype(f8), w1E.astype(bf), w2D.astype(f8),
            w2E.astype(f8), onesB.astype(f8), onesBE.astype(f8))


def _build_nc():
    nc = bacc.Bacc(None, target_bir_lowering=False, debug=False)

    x_ext = nc.declare_dram_parameter("x", [B_LOC, C, H, W], F32, isOutput=False)
    out_ext = nc.declare_dram_parameter("out", [B_LOC, C, H, W], F32, isOutput=True)
    w1A_ext = nc.declare_dram_parameter("w1A", [128, 2, 3, 128], FP8, isOutput=False)
    w1B_ext = nc.declare_dram_parameter("w1B", [128, 2, 3, 64], FP8, isOutput=False)
    w1E_ext = nc.declare_dram_parameter("w1E", [128, 3, 96], BF16, isOutput=False)
    w2D_ext = nc.declare_dram_parameter("w2D", [96, 2, 3, 32], FP8, isOutput=False)
    w2E_ext = nc.declare_dram_parameter("w2E", [96, 2, 80], FP8, isOutput=False)
    oB_ext = nc.declare_dram_parameter("onesB", [36, 2, 128], FP8, isOutput=False)
    oBE_ext = nc.declare_dram_parameter("onesBE", [36, 2, 2, 128], FP8, isOutput=False)

    with TileContext(nc) as tc:
        with (
            tc.tile_pool(name="wpool", bufs=1) as wpool,
            tc.tile_pool(name="xpool", bufs=1) as xpool,
            tc.tile_pool(name="hpool", bufs=1) as hpool,
            tc.tile_pool(name="opool", bufs=2) as opool,
            tc.tile_pool(name="scpool", bufs=4) as scpool,
            tc.tile_pool(name="smpool", bufs=4) as smpool,
            tc.tile_pool(name="tmpool", bufs=2) as tmpool,
            tc.tile_pool(name="psA", bufs=3, space="PSUM") as psA,
            tc.tile_pool(name="psB", bufs=2, space="PSUM") as psB,
            tc.tile_pool(name="psC", bufs=2, space="PSUM") as psC,
            tc.tile_pool(name="psE", bufs=1, space="PSUM") as psE,
        ):
            w1A = wpool.tile([128, 2, 3, 128], FP8)
            w1B = wpool.tile([128, 2, 3, 64], FP8)
            w1E = wpool.tile([128, 3, 96], BF16)
            w2D = wpool.tile([96, 2, 3, 32], FP8)
            w2E = wpool.tile([96, 2, 80], FP8)
            onesB = wpool.tile([36, 2, 128], FP8)
            onesBE = wpool.tile([36, 2, 2, 128], FP8)
            for dst, src in ((w1A, w1A_ext), (w1B, w1B_ext), (w1E, w1E_ext),
                             (w2D, w2D_ext), (w2E, w2E_ext), (onesB, oB_ext),
                             (onesBE, oBE_ext)):
                nc.sync.dma_start(out=dst[:], in_=src[:])

            x_sb = [xpool.tile([128, BUF_W], BF16, name=f"xsb{p}")
                    for p in range(PACKS)]
            x8 = xpool.tile([128, 2, BUF_W], FP8, name="x8")
            h64 = hpool.tile([64, BUF_W], FP8, name="h64")
            hrep4 = hpool.tile([96, 2, BUF_W], FP8, name="hrep4")
            A2 = hpool.tile([36, BUF_W], FP8, name="A2")

            def _yx(t, off, rows, cols=2, colstep=127):
                """[P, rows@W, cols@colstep] view at flat offset `off`."""
                base = t[:, off:off + 1]
                return AP(base.tensor, base.offset,
                          [list(base.ap[0]), [W, rows], [colstep, cols]])

            def guard_memsets():
                for p in range(PACKS):
                    nc.vector.memset(x_sb[p][:, 0:FG], 0.0)
                    nc.vector.memset(x_sb[p][:, FG + S:BUF_W], 0.0)
                nc.vector.memset(x8[:, :, 0:FG], 0.0)
                nc.vector.memset(x8[:, :, FG + S:BUF_W], 0.0)
                nc.gpsimd.memset(h64[:, 0:FG], 0.0)
                nc.gpsimd.memset(h64[:, FG + S:BUF_W], 0.0)
                nc.gpsimd.memset(hrep4[:, :, 0:FG], 0.0)
                nc.gpsimd.memset(hrep4[:, :, FG + S:BUF_W], 0.0)
                nc.gpsimd.memset(A2[0:4, 0:FG], 0.0)
                nc.gpsimd.memset(A2[0:4, FG + S:BUF_W], 0.0)

            def load(p, c):
                r1 = min(16 * c + 17, H)
                nc.gpsimd.dma_start(
                    out=x_sb[p][:, _pos(16 * c, 0):_pos(r1, 0)],
                    in_=x_ext[2 * p:2 * p + 2, :, 16 * c:r1]
                    .rearrange("b c h w -> (b c) (h w)"),
                )

            def cast8(p, c):
                r1 = min(16 * c + 17, H)
                a, b = _pos(16 * c, 0), _pos(r1, 0)
                nc.gpsimd.dma_start(out=x8[:, p, a:b], in_=x_sb[p][:, a:b])

            def conv1(v):
                # 16 rows = 4 psum tiles; weight reuse across pairs of tiles
                for g in range(2):
                    rts = (4 * v + 2 * g, 4 * v + 2 * g + 1)
                    pas = {rt: psA.tile([128, 512], F32, tag="psa", name="pa")
                           for rt in rts}
                    # w1A ky0 first (start clears the bank), w1B accumulates
                    # into [0:64], w1A ky1/ky2 last so stop closes the group
                    for rt in rts:
                        q = _pos(rt * 4 - 1, 0)
                        nc.tensor.matmul(
                            pas[rt][:, :], w1A[:, :, 0, :],
                            x8[:, :, q:q + 512],
                            perf_mode=mybir.MatmulPerfMode.DoubleRow,
                            start=True, stop=False)
                    for ky in range(3):
                        for rt in rts:
                            q = _pos(rt * 4 + ky - 1, 2)
                            nc.tensor.matmul(
                                pas[rt][0:64, 0:510], w1B[:, :, ky, :],
                                x8[:, :, q:q + 510],
                                perf_mode=mybir.MatmulPerfMode.DoubleRow,
                                start=False, stop=False)
                    for ky in (1, 2):
                        for rt in rts:
                            q = _pos(rt * 4 + ky - 1, 0)
                            nc.tensor.matmul(
                                pas[rt][:, :], w1A[:, :, ky, :],
                                x8[:, :, q:q + 512],
                                perf_mode=mybir.MatmulPerfMode.DoubleRow,
                                start=False, stop=(ky == 2))
                    for rt in rts:
                        q0 = _pos(rt * 4, 0)
                        pa = pas[rt]
                        t1 = scpool.tile([64, 512], BF16, tag="sc", name="t1")
                        nc.scalar.activation(t1[:, 0:510], pa[64:128, 1:511],
                                             AF.Copy)
                        nc.vector.tensor_add(t1[:, 0:510], pa[0:64, 0:510],
                                             t1[:, 0:510])
                        # interior columns only: edge cols are owned by
                        # conv1_edges (which may have run already)
                        hdst = AP(h64.tensor, q0 + 1,
                                  [list(h64[:, 0:1].ap[0]), [W, 4], [1, 126]])
                        tsrc = AP(t1.tensor, t1[:, 0:1].offset,
                                  [list(t1[:, 0:1].ap[0]), [W, 4], [1, 126]])
                        nc.vector.tensor_scalar_max(hdst, tsrc, 0.0)

            def conv1_edges(bb, p):
                # 32 rows, both sides: 3 bf16 MMs, K=128, M=96, N=128
                # rhs cols per row: {0, 1, 126, 127}; psum col j = 4y+u*2+v
                pe1 = psE.tile([96, 128], F32, tag="pse", name="pe1")
                for ky in range(3):
                    base = x_sb[p][:, _pos(32 * bb + ky - 1, 0):
                                   _pos(32 * bb + ky - 1, 0) + 1]
                    rhs = AP(base.tensor, base.offset,
                             [list(base.ap[0]), [W, 32], [126, 2], [1, 2]])
                    nc.tensor.matmul(pe1[:, :], w1E[:, ky, :], rhs,
                                     start=(ky == 0), stop=(ky == 2))
                # left[y] = blk0[4y] + blk1[4y+1]; right[y] = blk2[4y+2] + blk0[4y+3]
                te1 = smpool.tile([32, 128], BF16, tag="sm", name="te1")
                ts1 = smpool.tile([32, 32, 2], BF16, tag="sm", name="ts1")
                nc.scalar.activation(te1[:, :], pe1[0:32, :], AF.Copy)
                te1v = te1.rearrange("p (y r) -> p y r", r=4)
                pe1v1 = pe1[32:64, :].rearrange("p (y r) -> p y r", r=4)
                pe1v2 = pe1[64:96, :].rearrange("p (y r) -> p y r", r=4)
                nc.vector.tensor_add(ts1[:, :, 0], te1v[:, :, 0], pe1v1[:, :, 1])
                nc.vector.tensor_add(ts1[:, :, 1], te1v[:, :, 3], pe1v2[:, :, 2])
                nc.vector.tensor_scalar_max(
                    _yx(h64[32 * p:32 * p + 32], _pos(32 * bb, 0), 32),
                    ts1[:, :, :], 0.0)

            def repl_h(vv):
                a = max(0, _pos(32 * vv, 0) - W - 4)
                b = min(BUF_W, _pos(32 * vv + 32, 0) + W + 4)
                for s in range(2):
                    src = h64[32 * s:32 * s + 32]
                    a0 = max(a, W)
                    nc.sync.dma_start(out=hrep4[0:32, s, a0:b],
                                      in_=src[:, a0 - W:b - W])
                    nc.sync.dma_start(out=hrep4[32:64, s, a:b],
                                      in_=src[:, a:b])
                    b2 = min(b, BUF_W - W)
                    nc.sync.dma_start(out=hrep4[64:96, s, a:b2],
                                      in_=src[:, a + W:b2 + W])

            def conv2(v):
                for rt in range(4 * v, 4 * v + 4):
                    pz = psC.tile([32, 512], F32, tag="psc", name="pz")
                    for kx in range(3):
                        q = _pos(rt * 4, kx - 1)
                        nc.tensor.matmul(
                            pz[:, :], w2D[:, :, kx, :],
                            hrep4[:, :, q:q + 512],
                            perf_mode=mybir.MatmulPerfMode.DoubleRow,
                            start=(kx == 0), stop=(kx == 2))
                    q0 = _pos(rt * 4, 0)
                    nc.scalar.activation(A2[0:4, q0:q0 + 512], pz[0:4, :],
                                         AF.Sigmoid)

            def conv2_edges(bb):
                # one DR MM: K=192, M=12, rhs cols {0,1,126,127} x 32 rows
                pe2 = psE.tile([80, 4, 32], F32, tag="pse", name="pe2")
                for j, col in enumerate((0, 1, 126, 127)):
                    base = hrep4[:, 0, _pos(32 * bb, col):
                                 _pos(32 * bb, col) + 1]
                    rhs = AP(base.tensor, base.offset,
                             [list(base.ap[0]), [BUF_W, 2], [W, 32]])
                    nc.tensor.matmul(pe2[:, j, :], w2E[:, :, :], rhs,
                                     perf_mode=mybir.MatmulPerfMode.DoubleRow,
                                     start=(j == 0), stop=(j == 3))
                te2 = smpool.tile([4, 4, 32], BF16, tag="sm", name="te2")
                ts2 = smpool.tile([4, 32, 2], BF16, tag="sm", name="ts2")
                nc.scalar.activation(te2[:, :, :], pe2[0:4, :, :], AF.Copy)
                nc.vector.tensor_add(ts2[:, :, 0], te2[:, 0, :], pe2[32:36, 1, :])
                nc.vector.tensor_add(ts2[:, :, 1], te2[:, 3, :], pe2[64:68, 2, :])
                nc.scalar.activation(_yx(A2[0:4], _pos(32 * bb, 0), 32),
                                     ts2[:, :, :], AF.Sigmoid)

            def repl_a(bb):
                lo = max(0, _pos(32 * bb, 0) - 132)
                hi = _pos(32 * bb + 32, 0) if bb < 3 else FG + S
                for s in range(1, 9):
                    t = TAPS[s]
                    o = (t // 3 - 1) * W + (t % 3 - 1)
                    a = max(lo, -o)
                    nc.sync.dma_start(out=A2[4 * s:4 * s + 4, a:hi],
                                      in_=A2[0:4, a + o:hi + o])

            outs = {}

            def box(v):
                vv = v // 2
                if (vv, 0) not in outs:
                    for p in range(PACKS):
                        outs[(vv, p)] = opool.tile([128, 32 * W], BF16,
                                                   tag="ob", name=f"ob{p}")
                for p in range(PACKS):
                    o_sb = outs[(vv, p)]
                    for rt in range(4 * v, 4 * v + 4):
                        q0 = _pos(rt * 4, 0)
                        oo = (rt - 8 * vv) * 512
                        pb = psB.tile([128, 512], F32, tag="psb", name="pb")
                        nc.tensor.matmul(pb[:, :], onesB[:, p, :],
                                         A2[0:36, q0:q0 + 512],
                                         start=True, stop=True)
                        if rt % 2 == 0:
                            nc.vector.tensor_mul(
                                o_sb[:, oo:oo + 512],
                                x_sb[p][:, q0:q0 + 512], pb[:, :])
                        else:
                            tm = tmpool.tile([128, 512], BF16, tag="tm",
                                             name="tm")
                            nc.scalar.activation(tm[:, :], pb[:, :], AF.Copy)
                            nc.vector.tensor_mul(
                                o_sb[:, oo:oo + 512],
                                x_sb[p][:, q0:q0 + 512], tm[:, :])

            def box_edges(vv):
                for p in range(PACKS):
                    pbe = psE.tile([128, 2, 32], F32, tag="pse", name="pbe")
                    for e, col in enumerate((0, W - 1)):
                        base = A2[0:36, _pos(32 * vv, col):
                                  _pos(32 * vv, col) + 1]
                        rhs = AP(base.tensor, base.offset,
                                 [list(base.ap[0]), [W, 32]])
                        nc.tensor.matmul(pbe[:, e, :], onesBE[:, p, e, :],
                                         rhs, start=(e == 0), stop=(e == 1))
                    o_sb = outs[(vv, p)]
                    dst = o_sb[:, 0:32 * W].rearrange(
                        "p (y x) -> p y x", x=W)[:, :, 0:W:W - 1]
                    nc.vector.tensor_mul(
                        dst, _yx(x_sb[p], _pos(32 * vv, 0), 32),
                        pbe[:, :, :].rearrange("p e y -> p y e"))

            def store(vv, p):
                nc.gpsimd.dma_start(
                    out=out_ext[2 * p:2 * p + 2, :, 32 * vv:32 * vv + 32]
                    .rearrange("b c h w -> (b c) (h w)"),
                    in_=outs[(vv, p)][:, :],
                )

            # ---- emission: 16-row ticks ----
            guard_memsets()
            for c in (0, 1, 2):
                for p in range(PACKS):
                    load(p, c)
                    cast8(p, c)
            for u in range(1, 14):
                c = u + 2
                if c < 8:
                    for p in range(PACKS):
                        load(p, c)
                        cast8(p, c)
                if u % 2 == 1 and u <= 7:
                    bb = (u - 1) // 2
                    conv1_edges(bb, 0)
                    conv1_edges(bb, 1)
                v = u - 1
                if 0 <= v < 8:
                    conv1(v)
                if u % 2 == 1 and 3 <= u <= 9:
                    repl_h((u - 3) // 2)
                v = u - 3
                if 0 <= v < 8:
                    conv2(v)
                if u % 2 == 0 and 4 <= u <= 10:
                    conv2_edges((u - 4) // 2)
                if u % 2 == 0 and 6 <= u <= 12:
                    bb = (u - 6) // 2
                    repl_a(bb)
                    box(2 * bb)
                    box(2 * bb + 1)
                    box_edges(bb)
                if u % 2 == 1 and 7 <= u <= 13:
                    vv = (u - 7) // 2
                    store(vv, 0)
                    store(vv, 1)

    nc.compile()
    return nc


_CACHE = {}


def _get_nc():
    if "nc" not in _CACHE:
        _CACHE["nc"] = _build_nc()
    return _CACHE["nc"]


def _reset_device():
    try:
        import ctypes

        lib = ctypes.CDLL("/opt/axon/libaxon_pjrt.so")
        lib.axon_reset.restype = ctypes.c_int64
        lib.axon_reset()
    except Exception:
        pass


def _run(x, w1, w2, trace=False):
    x = np.ascontiguousarray(np.asarray(x, np.float32))
    w1A, w1B, w1E, w2D, w2E, onesB, onesBE = _host_weights(w1, w2)
    nc = _get_nc()
    in_maps = []
    for k in range(N_CORES):
        in_maps.append({
            "x": x[k * B_LOC:(k + 1) * B_LOC],
            "w1A": w1A, "w1B": w1B, "w1E": w1E, "w2D": w2D, "w2E": w2E,
            "onesB": onesB, "onesBE": onesBE,
        })
    try:
        res = run_bass_kernel_spmd(nc, in_maps, core_ids=list(range(N_CORES)),
                                   trace=trace)
    except Exception as e:
        if "unrecoverable" not in str(e).lower():
            raise
        _reset_device()
        res = run_bass_kernel_spmd(nc, in_maps, core_ids=list(range(N_CORES)),
                                   trace=trace)
    out = np.concatenate([r["out"] for r in res.results], axis=0)
    return out.astype(np.float32), res


def kernel(x, weights, w1, w2):
    out, _ = _run(x, w1, w2, trace=False)
    return out


def kernel_timed(x, weights, w1, w2):
    out, res = _run(x, w1, w2, trace=True)
    return out, res.exec_time_ns
